# revision 25
# baseline (speedup 1.0000x reference)
"""Decode-step GQA attention (bs=32, seq=1, 32 q heads / 8 kv heads, hd=128,
dim=4096, kv cache 2048) for 8 Trainium2 NeuronCores.

Sharding: tensor-parallel over heads. Core c owns kv head c and q heads
4c..4c+3: wq/wk/wv column-sharded, wo row-sharded, KV cache sharded on the
head axis. Each core computes a partial output projection; the host sums the
8 partials (no device collectives needed).

Device kernel layout choices:
  - K cache is staged host-side per core as K^T [b, hd, seq] in fp8-e3m4 so
    QK^T needs no on-device transpose and K DMA traffic halves; V stays
    natural [b, seq, hd] (optionally with a tail region in fp8).
  - scores are computed transposed ([seq, head] with seq on partitions) so
    exp runs on all 128 partitions; softmax denominators via a ones-vector
    matmul; normalization deferred to AFTER the PV matmul where it is a
    per-column scalar.
  - PV uses V as the *stationary* operand so the attention output lands
    pre-transposed [hd, head] in PSUM — no PE transposes of the attention
    rows, and wo consumes it directly.
  - The cache append (position start_pos) is handled by zeroing the stale
    position's exp weight and adding the new token's contribution as an extra
    PV accumulation term using a one-hot-masked outer product.
"""

import functools
import sys

import numpy as np

sys.path.insert(0, "/opt/trn_rl_repo")

import concourse.bass as bass  # noqa: E402
import concourse.tile as tile  # noqa: E402
from concourse import mybir  # noqa: E402
from concourse.bass_utils import run_bass_kernel_spmd  # noqa: E402

N_HEADS = 32
N_KV_HEADS = 8
HD = 128
DIM = 4096
BS = 32
MAXSEQ = 2048
NCORES = 8
HPC = N_HEADS // NCORES  # q heads per core (4)
QW = HPC * HD  # per-core wq width (512)
SCALE = 1.0 / float(np.sqrt(np.float32(HD)))

f32 = mybir.dt.float32
bf16 = mybir.dt.bfloat16
f8e3 = mybir.dt.float8e3

# dtype config: K^T cache fp8-e3m4; V bf16 with last V_F8 chunks in fp8.
KT_F8 = True
V_F8_CHUNKS = 0  # of the MAXSEQ//128 chunks, how many (from the end) in fp8


def _split_fat_waits(nc, max_waits=1):
    """walrus only encodes one semaphore wait per instruction; hoist extras
    onto preceding same-engine nops."""
    for f in nc.m.functions:
        for bb in f.blocks:
            new_list = []
            for ins in bb.instructions:
                si = ins.sync_info
                w = list(si.on_wait) if si and si.on_wait else []
                if len(w) > max_waits and ins.engine != mybir.EngineType.Unassigned:
                    extras, keep = w[:-max_waits], w[-max_waits:]
                    k = 0
                    while extras:
                        chunk, extras = extras[:max_waits], extras[max_waits:]
                        nop = mybir.InstNoOp(name=f"{ins.name}-wsplit{k}")
                        nop.engine = ins.engine
                        nop.sync_info = mybir.SyncInfo(on_wait=chunk, on_update=[])
                        new_list.append(nop)
                        k += 1
                    ins.sync_info.on_wait = keep
                new_list.append(ins)
            bb.instructions = new_list


def _build(start_pos):
    dkt = f8e3 if KT_F8 else bf16
    S = start_pos + 1  # attended sequence length
    NCH = (S + 127) // 128  # seq chunks
    NBF = NCH - V_F8_CHUNKS  # v chunks kept in bf16 (chunks [0, NBF))
    LC = start_pos // 128  # chunk holding the appended position
    G = 8  # batches per attention group

    nc = bass.Bass()
    xT = nc.declare_dram_parameter("xT", [128, DIM // 128, BS], bf16, isOutput=False)
    wqkv = nc.declare_dram_parameter("wqkv", [DIM, QW + 2 * HD], bf16, isOutput=False)
    wo = nc.declare_dram_parameter("wo", [QW, DIM], bf16, isOutput=False)
    kT = nc.declare_dram_parameter("kT", [BS, HD, MAXSEQ], dkt, isOutput=False)
    v_bf = nc.declare_dram_parameter("v_bf", [BS, 128, NBF * HD], bf16, isOutput=False)
    if V_F8_CHUNKS:
        v_f8 = nc.declare_dram_parameter(
            "v_f8", [BS, 128, (NCH - NBF) * HD], f8e3, isOutput=False
        )
    # f32 constants packed in one blob: iden | smask | cosq | sinq | cosk | sink
    CW = 128 + 1 + QW + QW + HD + HD
    cblob = nc.declare_dram_parameter("cblob", [128, CW], f32, isOutput=False)
    out = nc.declare_dram_parameter("out", [BS, DIM], f32, isOutput=True)

    NKCH = DIM // 128  # contraction chunks for the projections (32)

    with tile.TileContext(nc) as tc:
        with (
            tc.tile_pool(name="const", bufs=1) as const,
            tc.tile_pool(name="wpool", bufs=4) as wpool,
            tc.tile_pool(name="ktpool", bufs=18) as ktpool,
            tc.tile_pool(name="vpool", bufs=16) as vpool,
            tc.tile_pool(name="exppool", bufs=2) as exppool,
            tc.tile_pool(name="small", bufs=2) as small,
            tc.tile_pool(name="wopool", bufs=4) as wopool,
        ):
            # ---- constants (one blob DMA + xT) ----
            cblob_sb = const.tile([128, CW], f32)
            nc.sync.dma_start(out=cblob_sb[:], in_=cblob[:])
            o = 0
            iden_sb = cblob_sb[:, o : o + 128]; o += 128
            smask_sb = cblob_sb[:, o : o + 1]; o += 1
            cosq_sb = cblob_sb[:BS, o : o + QW]; o += QW
            sinq_sb = cblob_sb[:BS, o : o + QW]; o += QW
            cosk_sb = cblob_sb[:BS, o : o + HD]; o += HD
            sink_sb = cblob_sb[:BS, o : o + HD]; o += HD
            xT_sb = const.tile([128, NKCH, BS], bf16)
            nc.sync.dma_start(out=xT_sb[:], in_=xT[:])
            ones_bf = const.tile([128, 1], bf16)
            nc.vector.memset(ones_bf[:], 1.0)
            onesrow_sb = const.tile([1, 128], f32)
            nc.vector.memset(onesrow_sb[:], 1.0)

            qT_all = const.tile([128, HPC * BS], bf16)  # col = 32h + b
            attnT = const.tile([128, HPC * BS], bf16)  # col = 32h + b
            vnew_pad = const.tile([128, HD], bf16)
            e_new_pad = const.tile([128, HPC], f32)

            # ---- phase 1: QKV projections ----
            with tc.tile_pool(name="psum_p1", bufs=1, space="PSUM") as psum_p1:
                q_ps = psum_p1.tile([BS, QW], f32)
                k_ps = psum_p1.tile([BS, HD], f32)
                v_ps = psum_p1.tile([BS, HD], f32)
                for k in range(NKCH):
                    w_t = wpool.tile([128, QW + 2 * HD], bf16)
                    r = slice(128 * k, 128 * (k + 1))
                    nc.sync.dma_start(out=w_t[:], in_=wqkv[r, :])
                    st = k == 0
                    sp = k == NKCH - 1
                    lhsT = xT_sb[:, k, :]
                    nc.tensor.matmul(q_ps[:], lhsT, w_t[:, :QW], start=st, stop=sp)
                    nc.tensor.matmul(
                        k_ps[:], lhsT, w_t[:, QW : QW + HD], start=st, stop=sp
                    )
                    nc.tensor.matmul(
                        v_ps[:], lhsT, w_t[:, QW + HD :], start=st, stop=sp
                    )

                # ---- phase 2: rope, q transposes, new-token prep ----
                p2 = const  # single-use tiles, lifetime to end of kernel
                # rope(q)
                q_sw = p2.tile([BS, QW], f32)
                q_ps3 = q_ps[:].rearrange("p (i two) -> p i two", two=2)
                q_sw3 = q_sw[:].rearrange("p (i two) -> p i two", two=2)
                nc.vector.tensor_copy(out=q_sw3[:, :, 0], in_=q_ps3[:, :, 1])
                nc.vector.tensor_copy(out=q_sw3[:, :, 1], in_=q_ps3[:, :, 0])
                q_ro = p2.tile([BS, QW], f32)
                nc.vector.tensor_tensor(
                    q_ro[:], q_ps[:], cosq_sb, mybir.AluOpType.mult
                )
                nc.vector.tensor_tensor(
                    q_sw[:], q_sw[:], sinq_sb, mybir.AluOpType.mult
                )
                nc.vector.tensor_tensor(q_ro[:], q_ro[:], q_sw[:], mybir.AluOpType.add)
                # rope(k)
                k_sw = p2.tile([BS, HD], f32)
                k_ps3 = k_ps[:].rearrange("p (i two) -> p i two", two=2)
                k_sw3 = k_sw[:].rearrange("p (i two) -> p i two", two=2)
                nc.vector.tensor_copy(out=k_sw3[:, :, 0], in_=k_ps3[:, :, 1])
                nc.vector.tensor_copy(out=k_sw3[:, :, 1], in_=k_ps3[:, :, 0])
                k_ro = p2.tile([BS, HD], f32)
                nc.vector.tensor_tensor(
                    k_ro[:], k_ps[:], cosk_sb, mybir.AluOpType.mult
                )
                nc.vector.tensor_tensor(
                    k_sw[:], k_sw[:], sink_sb, mybir.AluOpType.mult
                )
                nc.vector.tensor_tensor(k_ro[:], k_ro[:], k_sw[:], mybir.AluOpType.add)
                # v_new (no rope)
                nc.vector.memset(vnew_pad[:], 0.0)
                nc.vector.tensor_copy(out=vnew_pad[:BS, :], in_=v_ps[:])

                # q^T assembly: qT_all[:, 32h + b] = q_ro[b, 128h + :]
                qT_v = qT_all[:].rearrange("p (h b) -> p h b", h=HPC)
                with tc.tile_pool(name="psum_t", bufs=2, space="PSUM") as psum_t:
                    for h in range(HPC):
                        ps_qt = psum_t.tile([128, BS], f32)
                        nc.tensor.transpose(
                            ps_qt[:], q_ro[:, 128 * h : 128 * (h + 1)], iden_sb[:BS, :BS]
                        )
                        nc.vector.tensor_copy(out=qT_v[:, h, :], in_=ps_qt[:])

                # s_new[b, h] = q_ro[b, 128h:] . k_ro[b, :]
                qk_new = p2.tile([BS, QW], f32)
                k_bc = k_ro[:, None, :].to_broadcast([BS, HPC, HD])
                nc.vector.tensor_tensor(
                    qk_new[:].rearrange("p (h d) -> p h d", h=HPC),
                    q_ro[:].rearrange("p (h d) -> p h d", h=HPC),
                    k_bc,
                    mybir.AluOpType.mult,
                )
                s_new = p2.tile([BS, HPC], f32)
                nc.vector.tensor_reduce(
                    out=s_new[:],
                    in_=qk_new[:].rearrange("p (h d) -> p h d", h=HPC),
                    axis=mybir.AxisListType.X,
                    op=mybir.AluOpType.add,
                )
                nc.vector.memset(e_new_pad[:], 0.0)
                nc.scalar.activation(
                    out=e_new_pad[:BS, :],
                    in_=s_new[:],
                    func=mybir.ActivationFunctionType.Exp,
                    scale=SCALE,
                )

            # ---- phase 3: attention, processed in pipelined batch groups ----
            GW = HPC * NCH  # scores width per batch (64)
            groups = [(0, 8), (8, 8), (16, 8), (24, 3), (27, 3), (30, 2)]
            with (
                tc.tile_pool(name="ps_sT", bufs=2, space="PSUM") as psA,
                tc.tile_pool(name="ps_atT", bufs=2, space="PSUM") as psB,
                tc.tile_pool(name="ps_den", bufs=1, space="PSUM") as psD,
                tc.tile_pool(name="ps_spec", bufs=1, space="PSUM") as psE,
                tc.tile_pool(name="ps_bc", bufs=1, space="PSUM") as psF,
            ):
                attnT_bv = attnT[:].rearrange("p (h b) -> p b h", h=HPC)
                qT_v2 = qT_all[:].rearrange("p (h b) -> p h b", h=HPC)

                def emit_post(st):
                    # work after exp(g): denominators, PV, normalize
                    b0, G, exp_g, emask_g, vbf_ts, vf8_ts = st
                    exp_v = exp_g[:].rearrange("p (B c h) -> p B c h", B=G, c=NCH)
                    ps_den = psD.tile([1, G * GW], f32)
                    nc.tensor.matmul(
                        ps_den[:], ones_bf[:], exp_g[:], start=True, stop=True
                    )
                    ps_spec = psE.tile([1, G * HPC], f32)
                    nc.tensor.matmul(
                        ps_spec[:], ones_bf[:], emask_g[:], start=True, stop=True
                    )
                    den16 = small.tile([1, G * HPC], f32, tag="den")
                    nc.vector.tensor_reduce(
                        out=den16[:],
                        in_=ps_den[:].rearrange("p (B c h) -> p B h c", B=G, c=NCH),
                        axis=mybir.AxisListType.X,
                        op=mybir.AluOpType.add,
                    )
                    nc.vector.tensor_tensor(
                        den16[:], den16[:], ps_spec[:], mybir.AluOpType.add
                    )
                    inv16 = small.tile([1, G * HPC], f32, tag="inv")
                    nc.vector.reciprocal(inv16[:], den16[:])
                    # PV with V stationary: out^T[hd, (b2, h)], unnormalized
                    ps_atT = psB.tile([128, G * HPC], f32)
                    for b2 in range(G):
                        sl = slice(HPC * b2, HPC * (b2 + 1))
                        for c in range(NCH):
                            cw = min(128, S - 128 * c)
                            v_l = (
                                vbf_ts[b2][:cw, c, :]
                                if c < NBF
                                else vf8_ts[b2][:cw, c - NBF, :]
                            )
                            nc.tensor.matmul(
                                ps_atT[:, sl],
                                v_l,
                                exp_v[:cw, b2, c, :],
                                start=(c == 0),
                                stop=False,
                            )
                        nc.tensor.matmul(
                            ps_atT[:, sl],
                            vnew_pad[:],
                            emask_g[:, sl],
                            start=False,
                            stop=True,
                        )
                    ps_bc = psF.tile([128, G * HPC], f32)
                    nc.tensor.matmul(
                        ps_bc[:], onesrow_sb[:], inv16[:], start=True, stop=True
                    )
                    inv_bc = small.tile([128, G * HPC], f32, tag="invbc")
                    nc.vector.tensor_copy(out=inv_bc[:], in_=ps_bc[:])
                    # normalize straight into attnT columns (b0..b0+G)
                    nc.vector.tensor_tensor(
                        attnT_bv[:, b0 : b0 + G, :],
                        ps_atT[:].rearrange("p (B h) -> p B h", B=G),
                        inv_bc[:].rearrange("p (B h) -> p B h", B=G),
                        mybir.AluOpType.mult,
                    )

                pending = None
                for gi, (b0, G) in enumerate(groups):
                    kt_ts, vbf_ts, vf8_ts = [], [], []
                    for b2 in range(G):
                        b = b0 + b2
                        kt_t = ktpool.tile([128, S], dkt, tag="kt")
                        nc.sync.dma_start(out=kt_t[:], in_=kT[b, :, :S])
                        vbf_t = vpool.tile([128, NBF, HD], bf16, tag="v")
                        nc.scalar.dma_start(out=vbf_t[:], in_=v_bf[b, :, : NBF * HD])
                        kt_ts.append(kt_t)
                        vbf_ts.append(vbf_t)
                        if V_F8_CHUNKS:
                            vf8_t = vpool.tile(
                                [128, NCH - NBF, HD], f8e3, tag="vf8"
                            )
                            nc.scalar.dma_start(
                                out=vf8_t[:], in_=v_f8[b, :, : (NCH - NBF) * HD]
                            )
                            vf8_ts.append(vf8_t)

                    # scores^T: [seq-in-chunk, (b2, c, h)]
                    ps_sT = psA.tile([128, G * GW], f32)
                    for b2 in range(G):
                        qT_b = qT_v2[:, :, b0 + b2]
                        for c in range(NCH):
                            cw = min(128, S - 128 * c)
                            o = GW * b2 + HPC * c
                            nc.tensor.matmul(
                                ps_sT[:cw, o : o + HPC],
                                kt_ts[b2][:, 128 * c : 128 * c + cw],
                                qT_b,
                                start=True,
                                stop=True,
                            )
                    exp_g = exppool.tile([128, G * GW], bf16, tag="exp")
                    nc.scalar.activation(
                        out=exp_g[:],
                        in_=ps_sT[:],
                        func=mybir.ActivationFunctionType.Exp,
                        scale=SCALE,
                    )
                    exp_v = exp_g[:].rearrange("p (B c h) -> p B c h", B=G, c=NCH)
                    nc.vector.tensor_tensor(
                        exp_v[:, :, LC, :],
                        exp_v[:, :, LC, :],
                        smask_sb[:, :, None].to_broadcast([128, G, HPC]),
                        mybir.AluOpType.mult,
                    )
                    emask_g = small.tile([128, G * HPC], bf16, tag="emask")
                    nc.vector.tensor_tensor(
                        emask_g[:].rearrange("p (B h) -> p B h", B=G),
                        e_new_pad[:, None, :].to_broadcast([128, G, HPC]),
                        iden_sb[:, b0 : b0 + G, None].to_broadcast([128, G, HPC]),
                        mybir.AluOpType.mult,
                    )
                    # wo prefetch: emitted after exp(g1) so the scalar engine
                    # issues it only once the early critical DMA is done
                    if gi == 1:
                        wo_tiles = []
                        for j in range(HPC):
                            wo_t = wopool.tile([128, DIM], bf16)
                            nc.scalar.dma_start(
                                out=wo_t[:], in_=wo[128 * j : 128 * (j + 1), :]
                            )
                            wo_tiles.append(wo_t)
                    if pending is not None:
                        emit_post(pending)
                    pending = (b0, G, exp_g, emask_g, vbf_ts, vf8_ts)
                emit_post(pending)

            # ---- phase 4: output projection ----
            NO = 8  # n-chunks of DIM/NO=512
            NW = DIM // NO
            attnT_v = attnT[:].rearrange("p (h b) -> p h b", h=HPC)
            with (
                tc.tile_pool(name="ps_o", bufs=4, space="PSUM") as psO,
                tc.tile_pool(name="out_sb", bufs=4) as outpool,
            ):
                for n in range(NO):
                    ps_o = psO.tile([BS, NW], f32)
                    ns = slice(NW * n, NW * (n + 1))
                    for j in range(HPC):
                        nc.tensor.matmul(
                            ps_o[:],
                            attnT_v[:, j, :],
                            wo_tiles[j][:, ns],
                            start=(j == 0),
                            stop=(j == HPC - 1),
                        )
                    o_sb = outpool.tile([BS, NW], f32, tag="osb")
                    if n % 2 == 0:
                        nc.vector.tensor_copy(out=o_sb[:], in_=ps_o[:])
                    else:
                        nc.scalar.activation(
                            out=o_sb[:],
                            in_=ps_o[:],
                            func=mybir.ActivationFunctionType.Copy,
                        )
                    nc.scalar.dma_start(out=out[:, ns], in_=o_sb[:])

    _split_fat_waits(nc)
    return nc


@functools.lru_cache(maxsize=8)
def _built(start_pos):
    return _build(start_pos)


def _host_prep(x, wq, wk, wv, wo, cache_k, cache_v, freqs_cos, freqs_sin, start_pos):
    import ml_dtypes

    f8np = ml_dtypes.float8_e3m4
    bfnp = ml_dtypes.bfloat16
    ktnp = f8np if KT_F8 else bfnp
    S = start_pos + 1
    NCH = (S + 127) // 128
    NBF = NCH - V_F8_CHUNKS
    x = np.ascontiguousarray(np.asarray(x, dtype=np.float32)).reshape(BS, DIM)
    wq = np.asarray(wq, dtype=np.float32)
    wk = np.asarray(wk, dtype=np.float32)
    wv = np.asarray(wv, dtype=np.float32)
    wo = np.asarray(wo, dtype=np.float32)
    cache_k = np.asarray(cache_k, dtype=np.float32)
    cache_v = np.asarray(cache_v, dtype=np.float32)
    cos = np.asarray(freqs_cos, dtype=np.float32).reshape(HD // 2)
    sin = np.asarray(freqs_sin, dtype=np.float32).reshape(HD // 2)

    # x^T chunks: xT[p, c, b] = x[b, 128c + p]
    xT = np.ascontiguousarray(
        x.reshape(BS, DIM // 128, 128).transpose(2, 1, 0).astype(bfnp)
    )

    cosF = np.empty(HD, np.float32)
    cosF[0::2] = cos
    cosF[1::2] = cos
    sinF = np.empty(HD, np.float32)
    sinF[0::2] = -sin
    sinF[1::2] = sin
    lc = start_pos // 128
    CW = 128 + 1 + QW + QW + HD + HD
    cblob = np.zeros((128, CW), np.float32)
    o = 0
    cblob[:, o : o + 128] = np.eye(128, dtype=np.float32)
    o += 128
    cblob[:, o] = (128 * lc + np.arange(128) < start_pos).astype(np.float32)
    o += 1
    cblob[:BS, o : o + QW] = np.tile(cosF, HPC)
    o += QW
    cblob[:BS, o : o + QW] = np.tile(sinF, HPC)
    o += QW
    cblob[:BS, o : o + HD] = cosF
    o += HD
    cblob[:BS, o : o + HD] = sinF
    o += HD

    in_maps = []
    for c in range(NCORES):
        # v chunk-major layout: [b, p, c*HD + d] = cache_v[b, 128c + p, head, d]
        v_nat = (
            cache_v[:, : 128 * NCH, c, :]
            .reshape(BS, NCH, 128, HD)
            .transpose(0, 2, 1, 3)
        )
        m = {
            "xT": xT,
            "wqkv": np.ascontiguousarray(
                np.concatenate(
                    [
                        wq[:, QW * c : QW * (c + 1)],
                        wk[:, HD * c : HD * (c + 1)],
                        wv[:, HD * c : HD * (c + 1)],
                    ],
                    axis=1,
                ).astype(bfnp)
            ),
            "wo": np.ascontiguousarray(wo[QW * c : QW * (c + 1), :].astype(bfnp)),
            "kT": np.ascontiguousarray(
                cache_k[:, :, c, :].transpose(0, 2, 1).astype(ktnp)
            ),
            "v_bf": np.ascontiguousarray(
                v_nat[:, :, :NBF, :].reshape(BS, 128, NBF * HD).astype(bfnp)
            ),
            "cblob": cblob,
        }
        if V_F8_CHUNKS:
            m["v_f8"] = np.ascontiguousarray(
                v_nat[:, :, NBF:, :].reshape(BS, 128, (NCH - NBF) * HD).astype(f8np)
            )
        in_maps.append(m)
    return in_maps


def kernel(
    x,
    wq,
    wk,
    wv,
    wo,
    cache_k,
    cache_v,
    freqs_cos,
    freqs_sin,
    start_pos,
    _trace=False,
    **_unused,
):
    sp = int(start_pos)
    nc = _built(sp)
    in_maps = _host_prep(
        x, wq, wk, wv, wo, cache_k, cache_v, freqs_cos, freqs_sin, sp
    )
    res = run_bass_kernel_spmd(nc, in_maps, list(range(NCORES)), trace=_trace)
    acc = np.zeros((BS, DIM), np.float32)
    for i in range(NCORES):
        acc += res.results[i]["out"]
    out = acc.reshape(BS, 1, DIM)
    if _trace:
        return out, res
    return out
